# revision 7
# baseline (speedup 1.0000x reference)
"""Causal self-attention Trainium2 kernel.

Full inputs in, full output out. Internally: 8 NeuronCores, data-parallel on
batch (2) x tensor-parallel on heads (4 groups of 4 heads). Each core computes
its 4 heads' attention for its batch in a transposed layout (head-dim /
key-dim on partitions) and a partial output projection; the host sums the 4
partial projections per batch and adds b_proj.

Per-core device program (all matmuls bf16 with fp32 PSUM accumulation):
  kqv^T = Wpacked.T @ x^T (+bias)       [768, 2048]   (k/q/v rows per head pair)
  per head: S^T = k^T.T-block @ q^T     [128m x 512n] blocks, causal-trimmed
            P^T = exp(S^T + addmask)    (no max subtraction; scores are O(1))
            U^T = [v|1].T-block @ P^T   rows 0-63 = unnormalized sa^T, row 64 = denom
            sa^T = U^T[0:64] * (1/denom broadcast)
  partial out^T = WprojT.T @ sa^T       [1024, 2048] fp32 -> DRAM
"""
import sys, os
sys.path.insert(0, '/opt/trn_rl_repo')
os.environ.setdefault("JAX_PLATFORMS", "")

import numpy as np
import ml_dtypes

import concourse.bass as bass
import concourse.bacc as bacc
import concourse.tile as tile
import concourse.mybir as mybir
from concourse import bass_utils

B, N, D, H, DH = 2, 2048, 1024, 16, 64
G = 4              # heads per core
NCORES = 8
NCH = 512          # n-chunk width
NJ = N // NCH      # 4 n-chunks
NMB = N // 128     # 16 m-blocks
EW = G * 3 * DH    # 768 packed kqv width per core
bf16 = ml_dtypes.bfloat16
f32 = np.float32
AF = mybir.ActivationFunctionType

_cache = {}


def _build_program():
    nc = bacc.Bacc("TRN2", target_bir_lowering=False, debug=False, num_devices=NCORES)

    xt_d = nc.dram_tensor("xt", [D, N], mybir.dt.bfloat16, kind="ExternalInput").ap()
    w_d = nc.dram_tensor("w", [D, EW], mybir.dt.bfloat16, kind="ExternalInput").ap()
    b_d = nc.dram_tensor("bvec", [EW // 128, 128, 1], mybir.dt.float32, kind="ExternalInput").ap()
    wpt_d = nc.dram_tensor("wpt", [2 * 128, D], mybir.dt.bfloat16, kind="ExternalInput").ap()
    mask_d = nc.dram_tensor("masks", [4, 128, NCH], mybir.dt.float32, kind="ExternalInput").ap()
    id_d = nc.dram_tensor("ident", [128, 128], mybir.dt.bfloat16, kind="ExternalInput").ap()
    out_d = nc.dram_tensor("outt", [D, N], mybir.dt.float32, kind="ExternalOutput").ap()

    with tile.TileContext(nc) as tc:
        _emit(nc, tc, xt_d, w_d, b_d, wpt_d, mask_d, id_d, out_d)

    nc.compile()
    return nc


def _emit(nc, tc, xt_d, w_d, b_d, wpt_d, mask_d, id_d, out_d):
    from contextlib import ExitStack

    dt = mybir.dt
    ctx = ExitStack()
    with ctx:
        consts = ctx.enter_context(tc.tile_pool(name="consts", bufs=1))
        work = ctx.enter_context(tc.tile_pool(name="work", bufs=1))

        # ---- constant loads ----
        xt_sb = []
        for dc in range(8):
            t = consts.tile([128, N], dt.bfloat16, name=f"xt{dc}", tag=f"xt{dc}")
            nc.sync.dma_start(t[:], xt_d[dc * 128:(dc + 1) * 128, :])
            xt_sb.append(t)
        w_sb = []
        for dc in range(8):
            t = consts.tile([128, EW], dt.bfloat16, name=f"w{dc}", tag=f"w{dc}")
            nc.sync.dma_start(t[:], w_d[dc * 128:(dc + 1) * 128, :])
            w_sb.append(t)
        b_sb = []
        for i in range(EW // 128):
            t = consts.tile([128, 1], dt.float32, name=f"b{i}", tag=f"b{i}")
            nc.sync.dma_start(t[:], b_d[i])
            b_sb.append(t)
        wpt_sb = []
        for kc in range(2):
            t = consts.tile([128, D], dt.bfloat16, name=f"wpt{kc}", tag=f"wpt{kc}")
            nc.sync.dma_start(t[:], wpt_d[kc * 128:(kc + 1) * 128, :])
            wpt_sb.append(t)
        mask_sb = []
        for r in range(4):
            t = consts.tile([128, NCH], dt.float32, name=f"mask{r}", tag=f"mask{r}")
            nc.sync.dma_start(t[:], mask_d[r])
            mask_sb.append(t)
        ident = consts.tile([128, 128], dt.bfloat16, name="ident", tag="ident")
        nc.sync.dma_start(ident[:], id_d[:])
        ones_sb = consts.tile([128, 64], dt.float32, name="ones", tag="ones")
        nc.gpsimd.memset(ones_sb[:], 1.0)

        # persistent kqv^T, v', sa^T buffers
        kqvT = [work.tile([128, N], dt.bfloat16, name=f"kqvT{i}", tag=f"kqvT{i}")
                for i in range(6)]
        vp = [work.tile([128, NMB, 66], dt.bfloat16, name=f"vp{h}", tag=f"vp{h}")
              for h in range(G)]
        saT = [work.tile([128, N], dt.bfloat16, name=f"saT{kc}", tag=f"saT{kc}")
               for kc in range(2)]

        # per-head slices (pair packing [k_e|k_o|q_e|q_o|v_e|v_o])
        def head_slices(h):
            p, o = h // 2, (h % 2) * 64
            kT = kqvT[3 * p][o:o + 64, :]
            qT = kqvT[3 * p + 1][o:o + 64, :]
            vT = kqvT[3 * p + 2][o:o + 64, :]
            return kT, qT, vT, o

        # ---- phase A: kqv projection + v transposes ----
        with tc.tile_pool(name="pk", bufs=3, space="PSUM") as pk, \
             tc.tile_pool(name="pt", bufs=2, space="PSUM") as pt:
            for p in range(2):
                for mcl in range(3):
                    mc = 3 * p + mcl
                    for j in range(NJ):
                        ps_t = pk.tile([128, NCH], dt.float32, tag="kqvpsum")
                        for dc in range(8):
                            nc.tensor.matmul(
                                ps_t[:],
                                w_sb[dc][:, mc * 128:(mc + 1) * 128],
                                xt_sb[dc][:, j * NCH:(j + 1) * NCH],
                                start=(dc == 0), stop=(dc == 7),
                            )
                        nc.scalar.activation(
                            kqvT[mc][:, j * NCH:(j + 1) * NCH], ps_t[:],
                            AF.Identity, bias=b_sb[mc][:],
                        )
                # v' build for this pair's two heads
                for h in (2 * p, 2 * p + 1):
                    _, _, vT_h, o = head_slices(h)
                    nc.gpsimd.memset(vp[h][:, :, 64:65], 1.0)
                    for mb in range(NMB):
                        tp = pt.tile([128, 64], dt.bfloat16, tag="vtp")
                        nc.tensor.transpose(
                            tp[:], vT_h[:, mb * 128:(mb + 1) * 128],
                            ident[o:o + 64, o:o + 64],
                        )
                        nc.vector.tensor_copy(vp[h][:, mb, 0:64], tp[:])

        # ---- phase B: attention + projection ----
        with tc.tile_pool(name="ps", bufs=3, space="PSUM") as ps, \
             tc.tile_pool(name="pu", bufs=2, space="PSUM") as pu, \
             tc.tile_pool(name="pp", bufs=2, space="PSUM") as pp, \
             tc.tile_pool(name="pbc", bufs=1, space="PSUM") as pbc, \
             tc.tile_pool(name="pP", bufs=4) as pPool, \
             tc.tile_pool(name="paux", bufs=2) as paux, \
             tc.tile_pool(name="pout", bufs=3) as pout:
            for j in range(NJ):
                nsl = slice(j * NCH, (j + 1) * NCH)
                for h in range(G):
                    kT, qT, vT, o = head_slices(h)
                    nm = 4 * (j + 1)
                    u_t = pu.tile([65, NCH], dt.float32, tag="u")

                    s_tiles = [None] * nm
                    p_tiles = [None] * nm
                    offs = [0] * nm

                    def emit_s(mi):
                        r = mi - 4 * j
                        off = 128 * r if r > 0 else 0
                        offs[mi] = off
                        s_t = ps.tile([128, NCH], dt.float32, tag="s")
                        nc.tensor.matmul(
                            s_t[:, off:],
                            kT[:, mi * 128:(mi + 1) * 128],
                            qT[:, j * NCH + off:(j + 1) * NCH],
                            start=True, stop=True,
                        )
                        if r >= 0:
                            nc.vector.tensor_add(
                                s_t[:, off:], s_t[:, off:], mask_sb[r][:, off:])
                        p_t = pPool.tile([128, NCH], dt.bfloat16, tag="p")
                        nc.scalar.activation(p_t[:, off:], s_t[:, off:], AF.Exp)
                        s_tiles[mi], p_tiles[mi] = s_t, p_t

                    def emit_pv(mi):
                        off = offs[mi]
                        nc.tensor.matmul(
                            u_t[:, off:],
                            vp[h][:, mi, 0:65],
                            p_tiles[mi][:, off:],
                            start=(mi == 0), stop=(mi == nm - 1),
                            skip_group_check=True,
                        )

                    # software-pipeline: keep PE 2 S-blocks ahead of PV
                    for mi in range(nm):
                        emit_s(mi)
                        if mi >= 2:
                            emit_pv(mi - 2)
                    emit_pv(max(nm - 2, 0))
                    if nm > 1:
                        emit_pv(nm - 1)

                    # normalize: recip of denominator row, matmul-broadcast
                    # across partitions (ones[1,64] outer rc[1,512]), multiply
                    rc = paux.tile([65, NCH], dt.float32, tag="rc")
                    nc.vector.reciprocal(rc[64:65, :], u_t[64:65, :])
                    bcp = pbc.tile([64, NCH], dt.float32, tag="bcp")
                    nc.tensor.matmul(bcp[:], ones_sb[64:65, 0:64], rc[64:65, :],
                                     start=True, stop=True)
                    bc = paux.tile([64, NCH], dt.float32, tag="bc")
                    nc.scalar.copy(bc[:], bcp[:])
                    kc, row = h // 2, (h % 2) * 64
                    if row == 0:
                        nc.vector.tensor_mul(saT[kc][0:64, nsl], u_t[0:64, :], bc[:])
                    else:
                        tmp = paux.tile([64, NCH], dt.bfloat16, tag="tmp")
                        nc.vector.tensor_mul(tmp[:], u_t[0:64, :], bc[:])
                        nc.sync.dma_start(saT[kc][64:128, nsl], tmp[:])

                # output projection for this n-chunk
                for oc in range(8):
                    pp_t = pp.tile([128, NCH], dt.float32, tag="pp")
                    for kc in range(2):
                        nc.tensor.matmul(
                            pp_t[:],
                            wpt_sb[kc][:, oc * 128:(oc + 1) * 128],
                            saT[kc][:, nsl],
                            start=(kc == 0), stop=(kc == 1),
                        )
                    o_t = pout.tile([128, NCH], dt.float32, tag="o")
                    nc.vector.tensor_copy(o_t[:], pp_t[:])
                    nc.sync.dma_start(out_d[oc * 128:(oc + 1) * 128, nsl], o_t[:])


def _host_prep(x, W_kqv, b_kqv, W_proj):
    """Build the 8 per-core input maps."""
    x = np.asarray(x, dtype=f32)
    W_kqv = np.asarray(W_kqv, dtype=f32)
    b_kqv = np.asarray(b_kqv, dtype=f32)
    W_proj = np.asarray(W_proj, dtype=f32)

    masks = np.zeros((4, 128, NCH), dtype=f32)
    mm = np.arange(128)[:, None]
    nn = np.arange(NCH)[None, :]
    for r in range(4):
        masks[r] = np.where(nn >= mm + 128 * r, 0.0, -10000.0)
    ident = np.eye(128, dtype=bf16)

    in_maps = []
    for c in range(NCORES):
        b, g = c // 4, c % 4
        heads = [4 * g + i for i in range(4)]
        # pack per pair: [k_e | k_o | q_e | q_o | v_e | v_o], q scaled by 1/8
        wcols, bcols = [], []
        for p in range(2):
            he, ho = heads[2 * p], heads[2 * p + 1]
            for sec in range(3):  # k, q, v
                scl = 0.125 if sec == 1 else 1.0
                for h in (he, ho):
                    wcols.append(W_kqv[h][:, sec * 64:(sec + 1) * 64] * scl)
                    bcols.append(b_kqv[h][sec * 64:(sec + 1) * 64] * scl)
        wpack = np.concatenate(wcols, axis=1)            # [1024, 768]
        bpack = np.concatenate(bcols).astype(f32)        # [768]
        in_maps.append({
            "xt": np.ascontiguousarray(x[b].T).astype(bf16),
            "w": wpack.astype(bf16),
            "bvec": bpack.reshape(EW // 128, 128, 1),
            "wpt": np.ascontiguousarray(W_proj[:, 256 * g:256 * (g + 1)].T).astype(bf16),
            "masks": masks,
            "ident": ident,
        })
    return in_maps


def run(x, W_kqv, b_kqv, W_proj, b_proj, trace=False, trace_cores=None):
    if "nc" not in _cache:
        _cache["nc"] = _build_program()
    nc = _cache["nc"]
    in_maps = _host_prep(x, W_kqv, b_kqv, W_proj)
    res = bass_utils.run_bass_kernel_spmd(
        nc, in_maps, core_ids=list(range(NCORES)),
        trace=trace, trace_cores=trace_cores,
    )
    b_proj = np.asarray(b_proj, dtype=f32)
    out = np.zeros((B, N, D), dtype=f32)
    for b in range(B):
        acc = res.results[4 * b]["outt"].astype(f32).copy()
        for g in range(1, 4):
            acc += res.results[4 * b + g]["outt"]
        out[b] = acc.T + b_proj[None, :]
    return out, res


def kernel(x, W_kqv, b_kqv, W_proj, b_proj):
    out, _ = run(x, W_kqv, b_kqv, W_proj, b_proj, trace=False)
    return out


# revision 20
# speedup vs baseline: 1.0726x; 1.0726x over previous
"""Causal self-attention Trainium2 kernel.

Full inputs in, full output out. Internally: 8 NeuronCores, data-parallel on
batch (2) x tensor-parallel on heads (4 groups of 4 heads). Each core computes
its 4 heads' attention for its batch in a transposed layout (head-dim /
key-dim on partitions) and a partial output projection; the host sums the 4
partial projections per batch and adds b_proj.

Per-core device program (all matmuls bf16 with fp32 PSUM accumulation):
  kqv^T = Wpacked.T @ x^T (+bias)       [768, 2048]   (k/q/v rows per head pair)
  per head: S^T = k^T.T-block @ q^T     [128m x 512n] blocks, causal-trimmed
            P^T = exp(S^T + addmask)    (no max subtraction; scores are O(1))
            U^T = [v|1].T-block @ P^T   rows 0-63 = unnormalized sa^T, row 64 = denom
            sa^T = U^T[0:64] * (1/denom broadcast)
  partial out^T = WprojT.T @ sa^T       [1024, 2048] fp32 -> DRAM
"""
import sys, os
sys.path.insert(0, '/opt/trn_rl_repo')
os.environ.setdefault("JAX_PLATFORMS", "")

import numpy as np
import ml_dtypes

import concourse.bass as bass
import concourse.bacc as bacc
import concourse.tile as tile
import concourse.mybir as mybir
from concourse import bass_utils

B, N, D, H, DH = 2, 2048, 1024, 16, 64
G = 4              # heads per core
NCORES = 8
NCH = 512          # n-chunk width
NJ = N // NCH      # 4 n-chunks
NMB = N // 128     # 16 m-blocks
EW = G * 3 * DH    # 768 packed kqv width per core
bf16 = ml_dtypes.bfloat16
f32 = np.float32
AF = mybir.ActivationFunctionType

_cache = {}


def _build_program():
    nc = bacc.Bacc("TRN2", target_bir_lowering=False, debug=False, num_devices=NCORES)

    xt_d = nc.dram_tensor("xt", [D, N], mybir.dt.bfloat16, kind="ExternalInput").ap()
    w_d = nc.dram_tensor("w", [D, EW], mybir.dt.bfloat16, kind="ExternalInput").ap()
    b_d = nc.dram_tensor("bvec", [EW // 128, 128, 1], mybir.dt.float32, kind="ExternalInput").ap()
    wpt_d = nc.dram_tensor("wpt", [2 * 128, D], mybir.dt.bfloat16, kind="ExternalInput").ap()
    mask_d = nc.dram_tensor("masks", [4, 128, NCH], mybir.dt.float32, kind="ExternalInput").ap()
    id_d = nc.dram_tensor("ident", [128, 128], mybir.dt.bfloat16, kind="ExternalInput").ap()
    ones_d = nc.dram_tensor("ones", [128, 64], mybir.dt.float32r, kind="ExternalInput").ap()
    out_d = nc.dram_tensor("outt", [D, N], mybir.dt.float32, kind="ExternalOutput").ap()

    with tile.TileContext(nc) as tc:
        _emit(nc, tc, xt_d, w_d, b_d, wpt_d, mask_d, id_d, ones_d, out_d)

    nc.compile()
    return nc


def _emit(nc, tc, xt_d, w_d, b_d, wpt_d, mask_d, id_d, ones_d, out_d):
    from contextlib import ExitStack

    dt = mybir.dt
    ctx = ExitStack()
    with ctx:
        consts = ctx.enter_context(tc.tile_pool(name="consts", bufs=1))
        work = ctx.enter_context(tc.tile_pool(name="work", bufs=1))

        # ---- constant loads (w/xt interleaved so the first kqv matmuls
        # can start as soon as chunk 0 of each has landed) ----
        xt_sb, w_sb = [], []
        for dc in range(8):
            tw = consts.tile([128, EW], dt.bfloat16, name=f"w{dc}", tag=f"w{dc}")
            nc.sync.dma_start(tw[:], w_d[dc * 128:(dc + 1) * 128, :])
            w_sb.append(tw)
            tx = consts.tile([128, N], dt.bfloat16, name=f"xt{dc}", tag=f"xt{dc}")
            nc.sync.dma_start(tx[:], xt_d[dc * 128:(dc + 1) * 128, :])
            xt_sb.append(tx)
        b_sb = []
        for i in range(EW // 128):
            t = consts.tile([128, 1], dt.float32, name=f"b{i}", tag=f"b{i}")
            nc.sync.dma_start(t[:], b_d[i])
            b_sb.append(t)
        wpt_sb = []
        for kc in range(2):
            t = consts.tile([128, D], dt.bfloat16, name=f"wpt{kc}", tag=f"wpt{kc}")
            nc.sync.dma_start(t[:], wpt_d[kc * 128:(kc + 1) * 128, :])
            wpt_sb.append(t)
        mask_sb = []
        for r in range(4):
            t = consts.tile([128, NCH], dt.float32, name=f"mask{r}", tag=f"mask{r}")
            nc.sync.dma_start(t[:], mask_d[r])
            mask_sb.append(t)
        ident = consts.tile([128, 128], dt.bfloat16, name="ident", tag="ident")
        nc.sync.dma_start(ident[:], id_d[:])
        ones_sb = consts.tile([128, 64], dt.float32r, name="ones", tag="ones")
        nc.sync.dma_start(ones_sb[:], ones_d[:])

        # persistent kqv^T, v', sa^T buffers
        kqvT = [work.tile([128, N], dt.bfloat16, name=f"kqvT{i}", tag=f"kqvT{i}")
                for i in range(6)]
        vp = [work.tile([128, NMB, 66], dt.bfloat16, name=f"vp{h}", tag=f"vp{h}")
              for h in range(G)]
        saT = [work.tile([128, N], dt.bfloat16, name=f"saT{kc}", tag=f"saT{kc}")
               for kc in range(2)]

        # per-head slices (pair packing [k_e|k_o|q_e|q_o|v_e|v_o])
        def head_slices(h):
            p, o = h // 2, (h % 2) * 64
            kT = kqvT[3 * p][o:o + 64, :]
            qT = kqvT[3 * p + 1][o:o + 64, :]
            vT = kqvT[3 * p + 2][o:o + 64, :]
            return kT, qT, vT, o

        # ---- phase A: kqv projection + v transposes ----
        with tc.tile_pool(name="pk", bufs=3, space="PSUM") as pk, \
             tc.tile_pool(name="pt", bufs=2, space="PSUM") as pt:
            for p in range(2):
                for mcl in range(3):
                    mc = 3 * p + mcl
                    for j in range(NJ):
                        ps_t = pk.tile([128, NCH], dt.float32, tag="kqvpsum")
                        for dc in range(8):
                            nc.tensor.matmul(
                                ps_t[:],
                                w_sb[dc][:, mc * 128:(mc + 1) * 128],
                                xt_sb[dc][:, j * NCH:(j + 1) * NCH],
                                start=(dc == 0), stop=(dc == 7),
                            )
                        # bias-add + bf16 cast on DVE (keeps ScalarE pure-Exp,
                        # avoiding activation-table reloads)
                        nc.vector.tensor_scalar_add(
                            kqvT[mc][:, j * NCH:(j + 1) * NCH], ps_t[:],
                            b_sb[mc][:],
                        )
                # v' build for this pair's two heads
                for h in (2 * p, 2 * p + 1):
                    _, _, vT_h, o = head_slices(h)
                    nc.gpsimd.memset(vp[h][:, :, 64:65], 1.0)
                    for mb in range(NMB):
                        tp = pt.tile([128, 64], dt.bfloat16, tag="vtp")
                        nc.tensor.transpose(
                            tp[:], vT_h[:, mb * 128:(mb + 1) * 128],
                            ident[o:o + 64, o:o + 64],
                        )
                        nc.vector.tensor_copy(vp[h][:, mb, 0:64], tp[:])

        # ---- phase B: attention + projection ----
        with tc.tile_pool(name="ps", bufs=3, space="PSUM") as ps, \
             tc.tile_pool(name="pu", bufs=2, space="PSUM") as pu, \
             tc.tile_pool(name="pp", bufs=2, space="PSUM") as pp, \
             tc.tile_pool(name="pbc", bufs=1, space="PSUM") as pbc, \
             tc.tile_pool(name="pP", bufs=4) as pPool, \
             tc.tile_pool(name="paux", bufs=2) as paux, \
             tc.tile_pool(name="pout", bufs=3) as pout:
            def emit_proj(j):
                nsl = slice(j * NCH, (j + 1) * NCH)
                for oc in range(8):
                    pp_t = pp.tile([128, NCH], dt.float32, tag="pp")
                    for kc in range(2):
                        nc.tensor.matmul(
                            pp_t[:],
                            wpt_sb[kc][:, oc * 128:(oc + 1) * 128],
                            saT[kc][:, nsl],
                            start=(kc == 0), stop=(kc == 1),
                        )
                    o_t = pout.tile([128, NCH], dt.float32, tag="o")
                    nc.vector.tensor_copy(o_t[:], pp_t[:])
                    nc.sync.dma_start(out_d[oc * 128:(oc + 1) * 128, nsl], o_t[:])

            for j in range(NJ):
                nsl = slice(j * NCH, (j + 1) * NCH)
                for h in range(G):
                    kT, qT, vT, o = head_slices(h)
                    nm = 4 * (j + 1)
                    u_t = pu.tile([65, NCH], dt.float32, tag="u")

                    s_tiles = [None] * nm
                    p_tiles = [None] * nm
                    offs = [0] * nm

                    def emit_s(mi):
                        r = mi - 4 * j
                        off = 128 * r if r > 0 else 0
                        offs[mi] = off
                        s_t = ps.tile([128, NCH], dt.float32, tag="s")
                        nc.tensor.matmul(
                            s_t[:, off:],
                            kT[:, mi * 128:(mi + 1) * 128],
                            qT[:, j * NCH + off:(j + 1) * NCH],
                            start=True, stop=True,
                        )
                        if r >= 0:
                            nc.vector.tensor_add(
                                s_t[:, off:], s_t[:, off:], mask_sb[r][:, off:])
                        p_t = pPool.tile([128, NCH], dt.bfloat16, tag="p")
                        nc.scalar.activation(p_t[:, off:], s_t[:, off:], AF.Exp)
                        s_tiles[mi], p_tiles[mi] = s_t, p_t

                    def emit_pv(mi):
                        off = offs[mi]
                        nc.tensor.matmul(
                            u_t[:, off:],
                            vp[h][:, mi, 0:65],
                            p_tiles[mi][:, off:],
                            start=(mi == 0), stop=(mi == nm - 1),
                            skip_group_check=True,
                        )

                    # software-pipeline: keep PE 3 S-blocks ahead of PV
                    depth = 3
                    for mi in range(nm):
                        emit_s(mi)
                        if mi >= depth:
                            emit_pv(mi - depth)
                    for mi in range(max(nm - depth, 0), nm):
                        emit_pv(mi)

                    # normalize: broadcast the denominator row across 64
                    # partitions with a K=1 f32r matmul, then DVE divide
                    # (no reciprocal: DVE RECIPROCAL is ~3.3us per row)
                    row_t = paux.tile([65, NCH], dt.float32r, tag="rc")
                    with nc.allow_low_precision(reason="softmax recip f32r for fast PE bcast"):
                        nc.vector.reciprocal(row_t[64:65, :], u_t[64:65, :])
                    bcp = pbc.tile([64, NCH], dt.float32, tag="bcp")
                    nc.tensor.matmul(bcp[:], ones_sb[64:65, 0:64],
                                     row_t[64:65, :],
                                     start=True, stop=True)
                    bc = paux.tile([64, NCH], dt.float32, tag="bc")
                    nc.vector.tensor_copy(bc[:], bcp[:])
                    kc, row = h // 2, (h % 2) * 64
                    if row == 0:
                        nc.vector.tensor_mul(saT[kc][0:64, nsl], u_t[0:64, :], bc[:])
                    else:
                        tmp = paux.tile([64, NCH], dt.bfloat16, tag="tmp")
                        nc.vector.tensor_mul(tmp[:], u_t[0:64, :], bc[:])
                        nc.sync.dma_start(saT[kc][64:128, nsl], tmp[:])

                # projection deferred one chunk: by the time PE reaches
                # proj(j-1) in its in-order stream, the normalize chains for
                # chunk j-1 finished during chunk j's S/PV work -> no stall.
                if j >= 1:
                    emit_proj(j - 1)
            emit_proj(NJ - 1)


def _host_prep(x, W_kqv, b_kqv, W_proj):
    """Build the 8 per-core input maps."""
    x = np.asarray(x, dtype=f32)
    W_kqv = np.asarray(W_kqv, dtype=f32)
    b_kqv = np.asarray(b_kqv, dtype=f32)
    W_proj = np.asarray(W_proj, dtype=f32)

    masks = np.zeros((4, 128, NCH), dtype=f32)
    mm = np.arange(128)[:, None]
    nn = np.arange(NCH)[None, :]
    for r in range(4):
        masks[r] = np.where(nn >= mm + 128 * r, 0.0, -10000.0)
    ident = np.eye(128, dtype=bf16)

    in_maps = []
    for c in range(NCORES):
        b, g = c // 4, c % 4
        heads = [4 * g + i for i in range(4)]
        # pack per pair: [k_e | k_o | q_e | q_o | v_e | v_o], q scaled by 1/8
        wcols, bcols = [], []
        for p in range(2):
            he, ho = heads[2 * p], heads[2 * p + 1]
            for sec in range(3):  # k, q, v
                scl = 0.125 if sec == 1 else 1.0
                for h in (he, ho):
                    wcols.append(W_kqv[h][:, sec * 64:(sec + 1) * 64] * scl)
                    bcols.append(b_kqv[h][sec * 64:(sec + 1) * 64] * scl)
        wpack = np.concatenate(wcols, axis=1)            # [1024, 768]
        bpack = np.concatenate(bcols).astype(f32)        # [768]
        in_maps.append({
            "xt": np.ascontiguousarray(x[b].T).astype(bf16),
            "w": wpack.astype(bf16),
            "bvec": bpack.reshape(EW // 128, 128, 1),
            "wpt": np.ascontiguousarray(W_proj[:, 256 * g:256 * (g + 1)].T).astype(bf16),
            "masks": masks,
            "ident": ident,
            "ones": np.ones((128, 64), dtype=f32),
        })
    return in_maps


def run(x, W_kqv, b_kqv, W_proj, b_proj, trace=False, trace_cores=None):
    if "nc" not in _cache:
        _cache["nc"] = _build_program()
    nc = _cache["nc"]
    in_maps = _host_prep(x, W_kqv, b_kqv, W_proj)
    res = bass_utils.run_bass_kernel_spmd(
        nc, in_maps, core_ids=list(range(NCORES)),
        trace=trace, trace_cores=trace_cores,
    )
    b_proj = np.asarray(b_proj, dtype=f32)
    out = np.zeros((B, N, D), dtype=f32)
    for b in range(B):
        acc = res.results[4 * b]["outt"].astype(f32).copy()
        for g in range(1, 4):
            acc += res.results[4 * b + g]["outt"]
        out[b] = acc.T + b_proj[None, :]
    return out, res


def kernel(x, W_kqv, b_kqv, W_proj, b_proj):
    out, _ = run(x, W_kqv, b_kqv, W_proj, b_proj, trace=False)
    return out


# revision 35
# speedup vs baseline: 1.1625x; 1.0837x over previous
"""Causal self-attention Trainium2 kernel.

Full inputs in, full output out. Internally: 8 NeuronCores, data-parallel on
batch (2) x tensor-parallel on heads (4 groups of 4 heads). Each core computes
its 4 heads' attention for its batch in a transposed layout (head-dim /
key-dim on partitions) and a partial output projection; the host sums the 4
partial projections per batch and adds b_proj.

Per-core device program (all matmuls bf16 with fp32 PSUM accumulation):
  kqv^T = Wpacked.T @ x^T (+bias)       [768, 2048]   (k/q/v rows per head pair)
  per head: S^T = k^T.T-block @ q^T     [128m x 512n] blocks, causal-trimmed
            P^T = exp(S^T + addmask)    (no max subtraction; scores are O(1))
            U^T = [v|1].T-block @ P^T   rows 0-63 = unnormalized sa^T, row 64 = denom
            sa^T = U^T[0:64] * (1/denom broadcast)
  partial out^T = WprojT.T @ sa^T       [1024, 2048] fp32 -> DRAM
"""
import sys, os
sys.path.insert(0, '/opt/trn_rl_repo')
os.environ.setdefault("JAX_PLATFORMS", "")

import numpy as np
import ml_dtypes

import concourse.bass as bass
import concourse.bacc as bacc
import concourse.tile as tile
import concourse.mybir as mybir
from concourse import bass_utils

B, N, D, H, DH = 2, 2048, 1024, 16, 64
G = 4              # heads per core
NCORES = 8
NCH = 512          # n-chunk width
NJ = N // NCH      # 4 n-chunks
NMB = N // 128     # 16 m-blocks
EW = G * 3 * DH    # 768 packed kqv width per core
bf16 = ml_dtypes.bfloat16
f32 = np.float32
AF = mybir.ActivationFunctionType

_cache = {}


def _build_program():
    nc = bacc.Bacc("TRN2", target_bir_lowering=False, debug=False, num_devices=NCORES)

    xt_d = nc.dram_tensor("xt", [D, N], mybir.dt.bfloat16, kind="ExternalInput").ap()
    w_d = nc.dram_tensor("w", [D, EW], mybir.dt.bfloat16, kind="ExternalInput").ap()
    b_d = nc.dram_tensor("bvec", [EW // 128, 128, 1], mybir.dt.float32, kind="ExternalInput").ap()
    wpt_d = nc.dram_tensor("wpt", [2 * 128, D], mybir.dt.bfloat16, kind="ExternalInput").ap()
    mask_d = nc.dram_tensor("masks", [4, 128, NCH], mybir.dt.bfloat16, kind="ExternalInput").ap()
    id_d = nc.dram_tensor("ident", [128, 128], mybir.dt.bfloat16, kind="ExternalInput").ap()
    ones_d = nc.dram_tensor("ones", [128, 64], mybir.dt.float32, kind="ExternalInput").ap()
    out_d = nc.dram_tensor("outt", [D, N], mybir.dt.float32, kind="ExternalOutput").ap()
    dbg = None
    if os.environ.get("KDBG") == "1":
        dbg = {
            "sa": nc.dram_tensor("dbg_sa", [2, 128, N], mybir.dt.bfloat16, kind="ExternalOutput").ap(),
            "kqvT": nc.dram_tensor("dbg_kqvT", [6, 128, N], mybir.dt.bfloat16, kind="ExternalOutput").ap(),
            "vp": nc.dram_tensor("dbg_vp", [4, 128, 16 * 66], mybir.dt.bfloat16, kind="ExternalOutput").ap(),
        }

    with tile.TileContext(nc) as tc:
        _emit(nc, tc, xt_d, w_d, b_d, wpt_d, mask_d, id_d, ones_d, out_d, dbg)

    nc.compile()
    return nc


def _emit(nc, tc, xt_d, w_d, b_d, wpt_d, mask_d, id_d, ones_d, out_d, dbg=None):
    from contextlib import ExitStack

    dt = mybir.dt
    ctx = ExitStack()
    with ctx:
        consts = ctx.enter_context(tc.tile_pool(name="consts", bufs=1))
        work = ctx.enter_context(tc.tile_pool(name="work", bufs=1))

        # ---- constant loads (w/xt interleaved so the first kqv matmuls
        # can start as soon as chunk 0 of each has landed) ----
        xt_sb, w_sb = [], []
        for dc in range(8):
            tw = consts.tile([128, EW], dt.bfloat16, name=f"w{dc}", tag=f"w{dc}")
            nc.sync.dma_start(tw[:], w_d[dc * 128:(dc + 1) * 128, :])
            w_sb.append(tw)
            tx = consts.tile([128, N], dt.bfloat16, name=f"xt{dc}", tag=f"xt{dc}")
            nc.sync.dma_start(tx[:], xt_d[dc * 128:(dc + 1) * 128, :])
            xt_sb.append(tx)
        b_sb = []
        for i in range(EW // 128):
            t = consts.tile([128, 1], dt.float32, name=f"b{i}", tag=f"b{i}")
            nc.sync.dma_start(t[:], b_d[i])
            b_sb.append(t)
        wpt_sb = []
        for kc in range(2):
            t = consts.tile([128, D], dt.bfloat16, name=f"wpt{kc}", tag=f"wpt{kc}")
            nc.sync.dma_start(t[:], wpt_d[kc * 128:(kc + 1) * 128, :])
            wpt_sb.append(t)
        mask_sb = []
        for r in range(4):
            t = consts.tile([128, NCH], dt.bfloat16, name=f"mask{r}", tag=f"mask{r}")
            nc.sync.dma_start(t[:], mask_d[r])
            mask_sb.append(t)
        ident = consts.tile([128, 128], dt.bfloat16, name="ident", tag="ident")
        nc.sync.dma_start(ident[:], id_d[:])
        ones_sb = consts.tile([128, 64], dt.float32, name="ones", tag="ones")
        nc.sync.dma_start(ones_sb[:], ones_d[:])

        # persistent kqv^T, v', sa^T buffers
        kqvT = [work.tile([128, N], dt.bfloat16, name=f"kqvT{i}", tag=f"kqvT{i}")
                for i in range(6)]
        vp = [work.tile([128, NMB, 66], dt.bfloat16, name=f"vp{h}", tag=f"vp{h}")
              for h in range(G)]
        saT = [work.tile([128, N], dt.bfloat16, name=f"saT{kc}", tag=f"saT{kc}")
               for kc in range(2)]

        # per-head slices (pair packing [k_e|k_o|q_e|q_o|v_e|v_o])
        def head_slices(h):
            p, o = h // 2, (h % 2) * 64
            kT = kqvT[3 * p][o:o + 64, :]
            qT = kqvT[3 * p + 1][o:o + 64, :]
            vT = kqvT[3 * p + 2][o:o + 64, :]
            return kT, qT, vT, o

        # ---- phase A: kqv projection + v transposes ----
        with tc.tile_pool(name="pk", bufs=3, space="PSUM") as pk, \
             tc.tile_pool(name="pt", bufs=2, space="PSUM") as pt:
            for p in range(2):
                for mcl in range(3):
                    mc = 3 * p + mcl
                    for j in range(NJ):
                        ps_t = pk.tile([128, NCH], dt.float32, tag="kqvpsum")
                        for dc in range(8):
                            nc.tensor.matmul(
                                ps_t[:],
                                w_sb[dc][:, mc * 128:(mc + 1) * 128],
                                xt_sb[dc][:, j * NCH:(j + 1) * NCH],
                                start=(dc == 0), stop=(dc == 7),
                            )
                        # bias-add + bf16 cast on ScalarE (idle during phase A)
                        nc.scalar.activation(
                            kqvT[mc][:, j * NCH:(j + 1) * NCH], ps_t[:],
                            AF.Identity, bias=b_sb[mc][:],
                        )
                # v' build for this pair's two heads
                for h in (2 * p, 2 * p + 1):
                    _, _, vT_h, o = head_slices(h)
                    nc.gpsimd.memset(vp[h][:, :, 64:65], 1.0)
                    for mb in range(NMB):
                        tp = pt.tile([128, 64], dt.bfloat16, tag="vtp")
                        nc.tensor.transpose(
                            tp[:], vT_h[:, mb * 128:(mb + 1) * 128],
                            ident[o:o + 64, o:o + 64],
                        )
                        nc.vector.tensor_copy(vp[h][:, mb, 0:64], tp[:])

        # ---- phase B: attention + projection ----
        # Heads are processed in pairs as two interleaved S->exp->mask->PV
        # chains so ScalarE/DVE latency of one head hides behind the other
        # head's PE work; proj matmuls of the previous chunk are scattered in
        # as further dependency-free PE filler. Keeps PE dense so the HAM
        # clock gate stays at 2.4GHz.
        with tc.tile_pool(name="ps", bufs=4, space="PSUM") as ps, \
             tc.tile_pool(name="pu", bufs=2, space="PSUM") as pu, \
             tc.tile_pool(name="pp", bufs=2, space="PSUM") as pp, \
             tc.tile_pool(name="pP", bufs=6) as pPool, \
             tc.tile_pool(name="paux", bufs=2) as paux, \
             tc.tile_pool(name="pout", bufs=3) as pout:
            def emit_proj_oc(j, oc):
                nsl = slice(j * NCH, (j + 1) * NCH)
                pp_t = pp.tile([128, NCH], dt.float32, tag="pp")
                for kc in range(2):
                    nc.tensor.matmul(
                        pp_t[:],
                        wpt_sb[kc][:, oc * 128:(oc + 1) * 128],
                        saT[kc][:, nsl],
                        start=(kc == 0), stop=(kc == 1),
                    )
                o_t = pout.tile([128, NCH], dt.float32, tag="o")
                nc.vector.tensor_copy(o_t[:], pp_t[:])
                nc.sync.dma_start(out_d[oc * 128:(oc + 1) * 128, nsl], o_t[:])

            for j in range(NJ):
                nsl = slice(j * NCH, (j + 1) * NCH)
                nm = 4 * (j + 1)
                for p in range(2):
                    pair = (2 * p, 2 * p + 1)
                    u_t = {h: pu.tile([65, NCH], dt.float32, tag="u", name=f"u{h}")
                           for h in pair}
                    p_tiles = {h: [None] * nm for h in pair}
                    offs = [0] * nm

                    def emit_s(h, mi):
                        kT, qT, _, _ = head_slices(h)
                        r = mi - 4 * j
                        off = 128 * r if r > 0 else 0
                        offs[mi] = off
                        s_t = ps.tile([128, NCH], dt.float32, tag="s")
                        nc.tensor.matmul(
                            s_t[:, off:],
                            kT[:, mi * 128:(mi + 1) * 128],
                            qT[:, j * NCH + off:(j + 1) * NCH],
                            start=True, stop=True,
                        )
                        p_t = pPool.tile([128, NCH], dt.bfloat16, tag="p")
                        if r >= 0:
                            # exp then 0/1-mask multiply (bf16 SBUF pair ->
                            # DVE 4x mode)
                            e_t = pPool.tile([128, NCH], dt.bfloat16, tag="e")
                            nc.scalar.activation(e_t[:, off:], s_t[:, off:], AF.Exp)
                            nc.vector.tensor_mul(
                                p_t[:, off:], e_t[:, off:], mask_sb[r][:, off:])
                        else:
                            nc.scalar.activation(p_t[:, off:], s_t[:, off:], AF.Exp)
                        p_tiles[h][mi] = p_t

                    def emit_pv(h, mi):
                        off = offs[mi]
                        nc.tensor.matmul(
                            u_t[h][:, off:],
                            vp[h][:, mi, 0:65],
                            p_tiles[h][mi][:, off:],
                            start=(mi == 0), stop=(mi == nm - 1),
                            skip_group_check=True,
                        )

                    depth = 2
                    for mi in range(nm):
                        for h in pair:
                            emit_s(h, mi)
                        if mi >= depth:
                            for h in pair:
                                emit_pv(h, mi - depth)
                    for mi in range(max(nm - depth, 0), nm):
                        for h in pair:
                            emit_pv(h, mi)

                    # normalize both heads: fast approx reciprocal of the
                    # denominator row, K=1 matmul broadcast, DVE multiply
                    for h in pair:
                        # reciprocal_approx_fast only works on full-width
                        # SBUF tiles at base partition 0 (PSUM or
                        # single-partition inputs misread on HW): broadcast
                        # the raw denominator row first, then take the
                        # reciprocal on the [64, n] broadcast tile.
                        dr_t = paux.tile([65, NCH], dt.float32, tag="dr")
                        nc.vector.tensor_copy(dr_t[64:65, :], u_t[h][64:65, :])
                        bcp = ps.tile([128, NCH], dt.float32, tag="s",
                                      name=f"bcp{h}")
                        nc.tensor.matmul(bcp[0:64, :], ones_sb[64:65, 0:64],
                                         dr_t[64:65, :], start=True, stop=True)
                        bc = paux.tile([64, NCH], dt.float32, tag="bc")
                        nc.vector.tensor_copy(bc[:], bcp[0:64, :])
                        rc64 = paux.tile([64, NCH], dt.float32, tag="rc64")
                        nc.vector.reciprocal_approx_fast(rc64[:], bc[:])
                        kc, row = h // 2, (h % 2) * 64
                        if row == 0:
                            nc.vector.tensor_mul(saT[kc][0:64, nsl],
                                                 u_t[h][0:64, :], rc64[:])
                        else:
                            tmp = paux.tile([64, NCH], dt.bfloat16, tag="tmp")
                            nc.vector.tensor_mul(tmp[:], u_t[h][0:64, :], rc64[:])
                            nc.sync.dma_start(saT[kc][64:128, nsl], tmp[:])

                    # scatter half of the previous chunk's projection after
                    # each pair as dependency-free PE filler
                    if j >= 1:
                        for oc in range(4 * p, 4 * p + 4):
                            emit_proj_oc(j - 1, oc)
            for oc in range(8):
                emit_proj_oc(NJ - 1, oc)
            if dbg is not None:
                for kc in range(2):
                    nc.sync.dma_start(dbg["sa"][kc], saT[kc][:])
                for i in range(6):
                    nc.sync.dma_start(dbg["kqvT"][i], kqvT[i][:])
                for h in range(4):
                    nc.sync.dma_start(dbg["vp"][h], vp[h].rearrange("p a b -> p (a b)"))


def _host_prep(x, W_kqv, b_kqv, W_proj):
    """Build the 8 per-core input maps."""
    x = np.asarray(x, dtype=f32)
    W_kqv = np.asarray(W_kqv, dtype=f32)
    b_kqv = np.asarray(b_kqv, dtype=f32)
    W_proj = np.asarray(W_proj, dtype=f32)

    masks = np.zeros((4, 128, NCH), dtype=bf16)
    mm = np.arange(128)[:, None]
    nn = np.arange(NCH)[None, :]
    for r in range(4):
        masks[r] = (nn >= mm + 128 * r).astype(bf16)
    ident = np.eye(128, dtype=bf16)

    in_maps = []
    for c in range(NCORES):
        b, g = c // 4, c % 4
        heads = [4 * g + i for i in range(4)]
        # pack per pair: [k_e | k_o | q_e | q_o | v_e | v_o], q scaled by 1/8
        wcols, bcols = [], []
        for p in range(2):
            he, ho = heads[2 * p], heads[2 * p + 1]
            for sec in range(3):  # k, q, v
                scl = 0.125 if sec == 1 else 1.0
                for h in (he, ho):
                    wcols.append(W_kqv[h][:, sec * 64:(sec + 1) * 64] * scl)
                    bcols.append(b_kqv[h][sec * 64:(sec + 1) * 64] * scl)
        wpack = np.concatenate(wcols, axis=1)            # [1024, 768]
        bpack = np.concatenate(bcols).astype(f32)        # [768]
        in_maps.append({
            "xt": np.ascontiguousarray(x[b].T).astype(bf16),
            "w": wpack.astype(bf16),
            "bvec": bpack.reshape(EW // 128, 128, 1),
            "wpt": np.ascontiguousarray(W_proj[:, 256 * g:256 * (g + 1)].T).astype(bf16),
            "masks": masks,
            "ident": ident,
            "ones": np.ones((128, 64), dtype=f32),
        })
    return in_maps


def run(x, W_kqv, b_kqv, W_proj, b_proj, trace=False, trace_cores=None):
    if "nc" not in _cache:
        _cache["nc"] = _build_program()
    nc = _cache["nc"]
    in_maps = _host_prep(x, W_kqv, b_kqv, W_proj)
    res = bass_utils.run_bass_kernel_spmd(
        nc, in_maps, core_ids=list(range(NCORES)),
        trace=trace, trace_cores=trace_cores,
    )
    b_proj = np.asarray(b_proj, dtype=f32)
    out = np.zeros((B, N, D), dtype=f32)
    for b in range(B):
        acc = res.results[4 * b]["outt"].astype(f32).copy()
        for g in range(1, 4):
            acc += res.results[4 * b + g]["outt"]
        out[b] = acc.T + b_proj[None, :]
    return out, res


def kernel(x, W_kqv, b_kqv, W_proj, b_proj):
    out, _ = run(x, W_kqv, b_kqv, W_proj, b_proj, trace=False)
    return out


# revision 44
# speedup vs baseline: 1.2877x; 1.1077x over previous
"""Causal self-attention Trainium2 kernel.

Full inputs in, full output out. Internally: 8 NeuronCores, data-parallel on
batch (2) x tensor-parallel on heads (4 groups of 4 heads). Each core computes
its 4 heads' attention for its batch in a transposed layout (head-dim /
key-dim on partitions) and a partial output projection; the host sums the 4
partial projections per batch and adds b_proj.

Per-core device program (all matmuls bf16 with fp32 PSUM accumulation):
  kqv^T = Wpacked.T @ x^T (+bias)       [768, 2048]   (k/q/v rows per head pair)
  per head: S^T = k^T.T-block @ q^T     [128m x 512n] blocks, causal-trimmed
            P^T = exp(S^T + addmask)    (no max subtraction; scores are O(1))
            U^T = [v|1].T-block @ P^T   rows 0-63 = unnormalized sa^T, row 64 = denom
            sa^T = U^T[0:64] * (1/denom broadcast)
  partial out^T = WprojT.T @ sa^T       [1024, 2048] fp32 -> DRAM
"""
import sys, os
sys.path.insert(0, '/opt/trn_rl_repo')
os.environ.setdefault("JAX_PLATFORMS", "")

import numpy as np
import ml_dtypes

import concourse.bass as bass
import concourse.bacc as bacc
import concourse.tile as tile
import concourse.mybir as mybir
from concourse import bass_utils

B, N, D, H, DH = 2, 2048, 1024, 16, 64
G = 4              # heads per core
NCORES = 8
NCH = 512          # n-chunk width
NJ = N // NCH      # 4 n-chunks
NMB = N // 128     # 16 m-blocks
EW = G * 3 * DH    # 768 packed kqv width per core
bf16 = ml_dtypes.bfloat16
f32 = np.float32
AF = mybir.ActivationFunctionType

_cache = {}


def _build_program():
    nc = bacc.Bacc("TRN2", target_bir_lowering=False, debug=False, num_devices=NCORES)

    xt_d = nc.dram_tensor("xt", [D, N], mybir.dt.bfloat16, kind="ExternalInput").ap()
    w_d = nc.dram_tensor("w", [D, EW], mybir.dt.bfloat16, kind="ExternalInput").ap()
    b_d = nc.dram_tensor("bvec", [EW // 128, 128, 1], mybir.dt.float32, kind="ExternalInput").ap()
    wpt_d = nc.dram_tensor("wpt", [2 * 128, D], mybir.dt.bfloat16, kind="ExternalInput").ap()
    mask_d = nc.dram_tensor("masks", [4, 128, NCH], mybir.dt.bfloat16, kind="ExternalInput").ap()
    id_d = nc.dram_tensor("ident", [128, 128], mybir.dt.bfloat16, kind="ExternalInput").ap()
    ones_d = nc.dram_tensor("ones", [128, 64], mybir.dt.float32, kind="ExternalInput").ap()
    out_d = nc.dram_tensor("outt", [D, N], mybir.dt.float32, kind="ExternalOutput").ap()
    dbg = None
    if os.environ.get("KDBG") == "1":
        dbg = {
            "sa": nc.dram_tensor("dbg_sa", [2, 128, N], mybir.dt.bfloat16, kind="ExternalOutput").ap(),
            "kqvT": nc.dram_tensor("dbg_kqvT", [6, 128, N], mybir.dt.bfloat16, kind="ExternalOutput").ap(),
            "vp": nc.dram_tensor("dbg_vp", [4, 128, 16 * 66], mybir.dt.bfloat16, kind="ExternalOutput").ap(),
        }

    with tile.TileContext(nc) as tc:
        _emit(nc, tc, xt_d, w_d, b_d, wpt_d, mask_d, id_d, ones_d, out_d, dbg)

    nc.compile()
    return nc


def _emit(nc, tc, xt_d, w_d, b_d, wpt_d, mask_d, id_d, ones_d, out_d, dbg=None):
    from contextlib import ExitStack

    dt = mybir.dt
    ctx = ExitStack()
    with ctx:
        consts = ctx.enter_context(tc.tile_pool(name="consts", bufs=1))
        work = ctx.enter_context(tc.tile_pool(name="work", bufs=1))

        # ---- constant loads (w/xt interleaved so the first kqv matmuls
        # can start as soon as chunk 0 of each has landed) ----
        xt_sb, w_sb = [], []
        for dc in range(8):
            tw = consts.tile([128, EW], dt.bfloat16, name=f"w{dc}", tag=f"w{dc}")
            nc.sync.dma_start(tw[:], w_d[dc * 128:(dc + 1) * 128, :])
            w_sb.append(tw)
            tx = consts.tile([128, N], dt.bfloat16, name=f"xt{dc}", tag=f"xt{dc}")
            nc.sync.dma_start(tx[:], xt_d[dc * 128:(dc + 1) * 128, :])
            xt_sb.append(tx)
        b_sb = []
        for i in range(EW // 128):
            t = consts.tile([128, 1], dt.float32, name=f"b{i}", tag=f"b{i}")
            nc.sync.dma_start(t[:], b_d[i])
            b_sb.append(t)
        wpt_sb = []
        for kc in range(2):
            t = consts.tile([128, D], dt.bfloat16, name=f"wpt{kc}", tag=f"wpt{kc}")
            nc.sync.dma_start(t[:], wpt_d[kc * 128:(kc + 1) * 128, :])
            wpt_sb.append(t)
        mask_sb = []
        for r in range(4):
            t = consts.tile([128, NCH], dt.bfloat16, name=f"mask{r}", tag=f"mask{r}")
            nc.sync.dma_start(t[:], mask_d[r])
            mask_sb.append(t)
        ident = consts.tile([128, 128], dt.bfloat16, name="ident", tag="ident")
        nc.sync.dma_start(ident[:], id_d[:])
        ones_sb = consts.tile([128, 64], dt.float32, name="ones", tag="ones")
        nc.sync.dma_start(ones_sb[:], ones_d[:])

        # persistent kqv^T, v', sa^T buffers
        kqvT = [work.tile([128, N], dt.bfloat16, name=f"kqvT{i}", tag=f"kqvT{i}")
                for i in range(6)]
        vp = [work.tile([128, NMB, 66], dt.bfloat16, name=f"vp{h}", tag=f"vp{h}")
              for h in range(G)]
        saT = [work.tile([128, N], dt.bfloat16, name=f"saT{kc}", tag=f"saT{kc}")
               for kc in range(2)]

        # per-head slices (pair packing [k_e|k_o|q_e|q_o|v_e|v_o])
        def head_slices(h):
            p, o = h // 2, (h % 2) * 64
            kT = kqvT[3 * p][o:o + 64, :]
            qT = kqvT[3 * p + 1][o:o + 64, :]
            vT = kqvT[3 * p + 2][o:o + 64, :]
            return kT, qT, vT, o

        # ---- fused emission ----
        # PE-bound kqv matmuls are overlapped with the ScalarE-exp-bound
        # attention windows: pair 0's kqv runs up front, pair 1's kqv groups
        # are scattered into pair 0's attention as PE filler; pair 1's
        # v-transposes and the output projection fill pair 1's attention
        # window. This keeps PE dense (HAM clock stays at 2.4GHz) and
        # balances PE vs ScalarE.
        # 8 PSUM banks, all pools open for the whole kernel (no mid-stream
        # pool swaps): ps(3, shared by S tiles / denom-broadcasts /
        # v-transposes) + pu(2) + pk(1) + pp(2). kqv psum groups alternate
        # between pk and pp (pp only carries projections in phase C, after
        # the last kqv group is long done).
        ps = ctx.enter_context(tc.tile_pool(name="ps", bufs=3, space="PSUM"))
        pu = ctx.enter_context(tc.tile_pool(name="pu", bufs=2, space="PSUM"))
        pk = ctx.enter_context(tc.tile_pool(name="pk", bufs=1, space="PSUM"))
        pp = ctx.enter_context(tc.tile_pool(name="pp", bufs=2, space="PSUM"))
        pPool = ctx.enter_context(tc.tile_pool(name="pP", bufs=6))
        paux = ctx.enter_context(tc.tile_pool(name="paux", bufs=2))
        pout = ctx.enter_context(tc.tile_pool(name="pout", bufs=3))
        kqv_ctr = [0]

        def emit_kqv_group(mc, jj, bias_on_dve):
            kqv_ctr[0] += 1
            if kqv_ctr[0] % 3 == 0:
                ps_t = pk.tile([128, NCH], dt.float32, tag="kqvpsum")
            else:
                ps_t = pp.tile([128, NCH], dt.float32, tag="pp", name="kqvp")
            for dc in range(8):
                nc.tensor.matmul(
                    ps_t[:],
                    w_sb[dc][:, mc * 128:(mc + 1) * 128],
                    xt_sb[dc][:, jj * NCH:(jj + 1) * NCH],
                    start=(dc == 0), stop=(dc == 7),
                )
            dst = kqvT[mc][:, jj * NCH:(jj + 1) * NCH]
            if bias_on_dve:
                nc.vector.tensor_scalar_add(dst, ps_t[:], b_sb[mc][:])
            else:
                nc.scalar.activation(dst, ps_t[:], AF.Identity, bias=b_sb[mc][:])

        def emit_vp_block(h, mb):
            _, _, vT_h, o = head_slices(h)
            tp = ps.tile([128, 64], dt.bfloat16, tag="s", name="vtp")
            nc.tensor.transpose(
                tp[:], vT_h[:, mb * 128:(mb + 1) * 128],
                ident[o:o + 64, o:o + 64],
            )
            nc.vector.tensor_copy(vp[h][:, mb, 0:64], tp[:])

        def emit_proj_oc(j, oc):
            nsl = slice(j * NCH, (j + 1) * NCH)
            pp_t = pp.tile([128, NCH], dt.float32, tag="pp")
            for kc in range(2):
                nc.tensor.matmul(
                    pp_t[:],
                    wpt_sb[kc][:, oc * 128:(oc + 1) * 128],
                    saT[kc][:, nsl],
                    start=(kc == 0), stop=(kc == 1),
                )
            o_t = pout.tile([128, NCH], dt.float32, tag="o")
            nc.vector.tensor_copy(o_t[:], pp_t[:])
            nc.sync.dma_start(out_d[oc * 128:(oc + 1) * 128, nsl], o_t[:])

        def emit_attn_chunk(j, p, fillers):
            """Attention for chunk j, head pair p, with `fillers` (zero-arg
            emitters of dependency-free PE work) spread across the m-loop."""
            nsl = slice(j * NCH, (j + 1) * NCH)
            nm = 4 * (j + 1)
            pair = (2 * p, 2 * p + 1)
            u_t = {h: pu.tile([65, NCH], dt.float32, tag="u", name=f"u{h}")
                   for h in pair}
            p_tiles = {h: [None] * nm for h in pair}
            offs = [0] * nm
            fill = list(fillers)
            # spread fillers over m-block boundaries (plus a tail flush)
            per_step = max(1, -(-len(fill) // max(nm, 1)))

            def emit_s(h, mi):
                kT, qT, _, _ = head_slices(h)
                r = mi - 4 * j
                off = 128 * r if r > 0 else 0
                offs[mi] = off
                s_t = ps.tile([128, NCH], dt.float32, tag="s")
                nc.tensor.matmul(
                    s_t[:, off:],
                    kT[:, mi * 128:(mi + 1) * 128],
                    qT[:, j * NCH + off:(j + 1) * NCH],
                    start=True, stop=True,
                )
                p_t = pPool.tile([128, NCH], dt.bfloat16, tag="p")
                if r >= 0:
                    e_t = pPool.tile([128, NCH], dt.bfloat16, tag="e")
                    nc.scalar.activation(e_t[:, off:], s_t[:, off:], AF.Exp)
                    nc.vector.tensor_mul(
                        p_t[:, off:], e_t[:, off:], mask_sb[r][:, off:])
                else:
                    nc.scalar.activation(p_t[:, off:], s_t[:, off:], AF.Exp)
                p_tiles[h][mi] = p_t

            def emit_pv(h, mi):
                off = offs[mi]
                nc.tensor.matmul(
                    u_t[h][:, off:],
                    vp[h][:, mi, 0:65],
                    p_tiles[h][mi][:, off:],
                    start=(mi == 0), stop=(mi == nm - 1),
                    skip_group_check=True,
                )

            depth = 2
            for mi in range(nm):
                for n_ in range(per_step):
                    if fill:
                        fill.pop(0)()
                for h in pair:
                    emit_s(h, mi)
                if mi >= depth:
                    for h in pair:
                        emit_pv(h, mi - depth)
            for mi in range(max(nm - depth, 0), nm):
                for h in pair:
                    emit_pv(h, mi)
            while fill:
                fill.pop(0)()

            # normalize both heads: PE-broadcast the raw denominator row,
            # then reciprocal_approx_fast on the [64, n] SBUF broadcast
            # (the only AP shape the custom op handles on HW), then multiply.
            for h in pair:
                dr_t = paux.tile([65, NCH], dt.float32, tag="dr")
                nc.vector.tensor_copy(dr_t[64:65, :], u_t[h][64:65, :])
                bcp = ps.tile([128, NCH], dt.float32, tag="s", name=f"bcp{h}")
                nc.tensor.matmul(bcp[0:64, :], ones_sb[64:65, 0:64],
                                 dr_t[64:65, :], start=True, stop=True)
                bc = paux.tile([64, NCH], dt.float32, tag="bc")
                nc.vector.tensor_copy(bc[:], bcp[0:64, :])
                rc64 = paux.tile([64, NCH], dt.float32, tag="rc64")
                nc.vector.reciprocal_approx_fast(rc64[:], bc[:])
                kc, row = h // 2, (h % 2) * 64
                if row == 0:
                    nc.vector.tensor_mul(saT[kc][0:64, nsl],
                                         u_t[h][0:64, :], rc64[:])
                else:
                    tmp = paux.tile([64, NCH], dt.bfloat16, tag="tmp")
                    nc.vector.tensor_mul(tmp[:], u_t[h][0:64, :], rc64[:])
                    nc.sync.dma_start(saT[kc][64:128, nsl], tmp[:])

        # A0: kqv pair 0 + v' for heads 0/1
        for h in range(G):
            nc.gpsimd.memset(vp[h][:, :, 64:65], 1.0)
        for mc in (0, 1, 2):
            for jj in range(NJ):
                emit_kqv_group(mc, jj, bias_on_dve=False)
        for mb in range(NMB):
            emit_vp_block(0, mb)
            emit_vp_block(1, mb)

        # B0: attention pair 0, with pair 1's kqv scattered in (jj-major so
        # pair 1's chunk-j inputs are ready a full chunk ahead)
        for j in range(NJ):
            fillers = [
                (lambda mc=mc, jj=j: emit_kqv_group(mc, jj, bias_on_dve=True))
                for mc in (3, 4, 5)
            ]
            emit_attn_chunk(j, 0, fillers)

        # C: attention pair 1, with JIT v-transposes and the projection of
        # finished chunks scattered in
        for j in range(NJ):
            fillers = []
            for mb in range(4 * j, 4 * j + 4):
                fillers.append(lambda mb=mb: emit_vp_block(2, mb))
                fillers.append(lambda mb=mb: emit_vp_block(3, mb))
            if j >= 1:
                fillers += [(lambda oc=oc, jj=j - 1: emit_proj_oc(jj, oc))
                            for oc in range(8)]
            emit_attn_chunk(j, 1, fillers)
        for oc in range(8):
            emit_proj_oc(NJ - 1, oc)

        if dbg is not None:
            for kc in range(2):
                nc.sync.dma_start(dbg["sa"][kc], saT[kc][:])
            for i in range(6):
                nc.sync.dma_start(dbg["kqvT"][i], kqvT[i][:])
            for h in range(4):
                nc.sync.dma_start(dbg["vp"][h], vp[h].rearrange("p a b -> p (a b)"))


def _host_prep(x, W_kqv, b_kqv, W_proj):
    """Build the 8 per-core input maps."""
    x = np.asarray(x, dtype=f32)
    W_kqv = np.asarray(W_kqv, dtype=f32)
    b_kqv = np.asarray(b_kqv, dtype=f32)
    W_proj = np.asarray(W_proj, dtype=f32)

    masks = np.zeros((4, 128, NCH), dtype=bf16)
    mm = np.arange(128)[:, None]
    nn = np.arange(NCH)[None, :]
    for r in range(4):
        masks[r] = (nn >= mm + 128 * r).astype(bf16)
    ident = np.eye(128, dtype=bf16)

    in_maps = []
    for c in range(NCORES):
        b, g = c // 4, c % 4
        heads = [4 * g + i for i in range(4)]
        # pack per pair: [k_e | k_o | q_e | q_o | v_e | v_o], q scaled by 1/8
        wcols, bcols = [], []
        for p in range(2):
            he, ho = heads[2 * p], heads[2 * p + 1]
            for sec in range(3):  # k, q, v
                scl = 0.125 if sec == 1 else 1.0
                for h in (he, ho):
                    wcols.append(W_kqv[h][:, sec * 64:(sec + 1) * 64] * scl)
                    bcols.append(b_kqv[h][sec * 64:(sec + 1) * 64] * scl)
        wpack = np.concatenate(wcols, axis=1)            # [1024, 768]
        bpack = np.concatenate(bcols).astype(f32)        # [768]
        in_maps.append({
            "xt": np.ascontiguousarray(x[b].T).astype(bf16),
            "w": wpack.astype(bf16),
            "bvec": bpack.reshape(EW // 128, 128, 1),
            "wpt": np.ascontiguousarray(W_proj[:, 256 * g:256 * (g + 1)].T).astype(bf16),
            "masks": masks,
            "ident": ident,
            "ones": np.ones((128, 64), dtype=f32),
        })
    return in_maps


def run(x, W_kqv, b_kqv, W_proj, b_proj, trace=False, trace_cores=None):
    if "nc" not in _cache:
        _cache["nc"] = _build_program()
    nc = _cache["nc"]
    in_maps = _host_prep(x, W_kqv, b_kqv, W_proj)
    res = bass_utils.run_bass_kernel_spmd(
        nc, in_maps, core_ids=list(range(NCORES)),
        trace=trace, trace_cores=trace_cores,
    )
    b_proj = np.asarray(b_proj, dtype=f32)
    out = np.zeros((B, N, D), dtype=f32)
    for b in range(B):
        acc = res.results[4 * b]["outt"].astype(f32).copy()
        for g in range(1, 4):
            acc += res.results[4 * b + g]["outt"]
        out[b] = acc.T + b_proj[None, :]
    return out, res


def kernel(x, W_kqv, b_kqv, W_proj, b_proj):
    out, _ = run(x, W_kqv, b_kqv, W_proj, b_proj, trace=False)
    return out
